# revision 7
# baseline (speedup 1.0000x reference)
"""Trainium2 Bass kernel for nn_LocalDictionaryLoss.

Math: with z = x @ A  ([B, D]), the loss
    a = 0.5 * mean_b ||y_b - z_b||^2
    b = mean_b sum_k ||y_b - A_k||^2 * x[b,k]
collapses (expanding ||y_b - A_k||^2 = y_sq[b] - 2 y_b.A_k + A_sq[k]) to
    loss = (1/B) * sum_b 0.5*(y_sq[b] + z_sq[b] - 2*yz[b])
         + (0.1/B) * sum_b (y_sq[b]*sx[b] + xA_sq[b] - 2*yz[b])
with per-row scalars
    y_sq[b] = ||y_b||^2, z_sq[b] = ||z_b||^2, yz[b] = y_b.z_b,
    sx[b] = sum_k x[b,k], xA_sq[b] = sum_k x[b,k]*A_sq[k].
So the [B,K] "weight" GEMM is never materialized: one [B,K]x[K,D] GEMM + two
extra moving columns (A_sq, ones) appended to A give everything.

Sharding: batch across 8 cores (1024 rows each), A replicated.
GEMM runs in bf16 (inputs host-cast), all accumulation fp32 (PSUM / ACT / DVE).
"""
import sys

sys.path.insert(0, "/opt/trn_rl_repo")
from contextlib import ExitStack

import ml_dtypes
import numpy as np

import concourse.bass as bass
import concourse.tile as tile
from concourse import bacc, mybir
from concourse import bass_utils
from concourse._compat import with_exitstack

f32 = mybir.dt.float32
bf16 = mybir.dt.bfloat16
AF = mybir.ActivationFunctionType
ALU = mybir.AluOpType

P = 128
B, K, D = 8192, 2048, 1024
NCORES = 8
BSH = B // NCORES          # 1024 batch rows per core
KT = K // P                # 16 k-tiles
MT = BSH // P              # 8 m-tiles
EX = D + 4                 # A cols + [A_sq, ones, pad, pad] (8B-aligned stride)
PENALTY = 0.1

_COMPILED = {}


@with_exitstack
def _loss_kernel(ctx: ExitStack, tc: tile.TileContext, out_ap, xt_ap, y_ap, a_ap):
    nc = tc.nc
    resident = ctx.enter_context(tc.tile_pool(name="resident", bufs=1))
    scr_pool = ctx.enter_context(tc.tile_pool(name="scr", bufs=4))
    stats = ctx.enter_context(tc.tile_pool(name="stats", bufs=1))
    psum = ctx.enter_context(tc.tile_pool(name="psum", bufs=2, space="PSUM"))

    a_sb = resident.tile([P, KT * EX], bf16, name="a_sb")
    xt_sb = resident.tile([P, KT * D], bf16, name="xt_sb")
    y_sb = resident.tile([P, MT * D], f32, name="y_sb")
    asq_f = stats.tile([P, KT], f32, name="asq_f")

    stat_zsq = stats.tile([P, MT], f32, name="stat_zsq")
    stat_yz = stats.tile([P, MT], f32, name="stat_yz")
    stat_ysq = stats.tile([P, MT], f32, name="stat_ysq")
    stat_sx = stats.tile([P, MT], f32, name="stat_sx")
    stat_zsq1 = stats.tile([P, MT], f32, name="stat_zsq1")
    stat_yz1 = stats.tile([P, MT], f32, name="stat_yz1")
    stat_xasq = stats.tile([P, MT], f32, name="stat_xasq")

    # ---- loads (interleaved a/xt so early m-tiles can start; full-tile
    # DMAs — column-block splitting costs more in per-DMA overhead than the
    # overlap it buys, per the cost-model timeline) ----
    for t in range(KT):
        nc.sync.dma_start(a_sb[:, t * EX:t * EX + D], a_ap[t * P:(t + 1) * P, :])
        nc.sync.dma_start(xt_sb[:, t * D:(t + 1) * D], xt_ap[t * P:(t + 1) * P, :])
    for m in range(MT):
        nc.sync.dma_start(y_sb[:, m * D:(m + 1) * D], y_ap[m * P:(m + 1) * P, :])

    # ---- A_sq + ones columns (ACT square-accumulate, then cast to bf16) ----
    for t in range(KT):
        scr = scr_pool.tile([P, D], bf16, name=f"scr_asq{t}", tag="scr")
        nc.scalar.activation(scr[:], a_sb[:, t * EX:t * EX + D], AF.Square,
                             accum_out=asq_f[:, t:t + 1])
        nc.vector.tensor_copy(a_sb[:, t * EX + D:t * EX + D + 1], asq_f[:, t:t + 1])
        nc.vector.memset(a_sb[:, t * EX + D + 1:t * EX + D + 4], 1.0)

    # ---- main GEMM: z[m-tile] = x_shard @ [A | A_sq | 1] ----
    for m in range(MT):
        pz = psum.tile([P, EX], f32, name=f"pz{m}", tag="pz")
        for t in range(KT):
            lhsT = xt_sb[:, t * D + m * P:t * D + (m + 1) * P]
            st, sp = (t == 0), (t == KT - 1)
            nc.tensor.matmul(pz[:, 0:512], lhsT, a_sb[:, t * EX:t * EX + 512],
                             start=st, stop=sp)
            nc.tensor.matmul(pz[:, 512:1024], lhsT, a_sb[:, t * EX + 512:t * EX + 1024],
                             start=st, stop=sp)
            nc.tensor.matmul(pz[:, 1024:1028], lhsT, a_sb[:, t * EX + 1024:t * EX + 1028],
                             start=st, stop=sp)

        # ---- per-m epilogue: evacuate PSUM per-bank, reduce from SBUF ----
        y_m = y_sb[:, m * D:(m + 1) * D]
        z0 = scr_pool.tile([P, 512], f32, name=f"z0_{m}", tag="zev")
        z1 = scr_pool.tile([P, 512], f32, name=f"z1_{m}", tag="zev")
        nc.vector.tensor_copy(z0[:], pz[:, 0:512])
        nc.vector.tensor_copy(z1[:], pz[:, 512:1024])
        exv = scr_pool.tile([P, 4], f32, name=f"ex_{m}", tag="exv")
        nc.vector.tensor_copy(exv[:], pz[:, 1024:1028])
        s0 = scr_pool.tile([P, 512], bf16, name=f"s0_{m}", tag="scr")
        nc.scalar.activation(s0[:], z0[:], AF.Square,
                             accum_out=stat_zsq[:, m:m + 1])
        s1 = scr_pool.tile([P, 512], bf16, name=f"s1_{m}", tag="scr")
        nc.scalar.activation(s1[:], z1[:], AF.Square,
                             accum_out=stat_zsq1[:, m:m + 1])
        s2 = scr_pool.tile([P, 512], f32, name=f"s2_{m}", tag="scrf")
        nc.vector.tensor_mul(s2[:], y_m[:, 0:512], z0[:])
        nc.vector.tensor_reduce(stat_yz[:, m:m + 1], s2[:],
                                axis=mybir.AxisListType.X, op=ALU.add)
        s3 = scr_pool.tile([P, 512], f32, name=f"s3_{m}", tag="scrf")
        nc.vector.tensor_mul(s3[:], y_m[:, 512:1024], z1[:])
        nc.vector.tensor_reduce(stat_yz1[:, m:m + 1], s3[:],
                                axis=mybir.AxisListType.X, op=ALU.add)
        s4 = scr_pool.tile([P, D], bf16, name=f"s4_{m}", tag="scry")
        nc.scalar.activation(s4[:], y_m, AF.Square,
                             accum_out=stat_ysq[:, m:m + 1])
        nc.vector.tensor_copy(stat_xasq[:, m:m + 1], exv[:, 0:1])
        nc.vector.tensor_copy(stat_sx[:, m:m + 1], exv[:, 1:2])

    # ---- combine: L = 0.5*(ysq+zsq) - 1.2*yz + 0.1*ysq*sx + 0.1*xasq ----
    zs = stats.tile([P, MT], f32, name="zs")
    nc.vector.tensor_add(zs[:], stat_zsq[:], stat_zsq1[:])
    yzt = stats.tile([P, MT], f32, name="yzt")
    nc.vector.tensor_add(yzt[:], stat_yz[:], stat_yz1[:])
    c1 = stats.tile([P, MT], f32, name="c1")
    nc.vector.tensor_add(c1[:], stat_ysq[:], zs[:])
    c2 = stats.tile([P, MT], f32, name="c2")
    nc.vector.scalar_tensor_tensor(c2[:], in0=yzt[:], scalar=-2.4,
                                   in1=c1[:], op0=ALU.mult, op1=ALU.add)
    c3 = stats.tile([P, MT], f32, name="c3")
    nc.vector.tensor_mul(c3[:], stat_ysq[:], stat_sx[:])
    c4 = stats.tile([P, MT], f32, name="c4")
    nc.vector.scalar_tensor_tensor(c4[:], in0=c3[:], scalar=0.2,
                                   in1=c2[:], op0=ALU.mult, op1=ALU.add)
    c5 = stats.tile([P, MT], f32, name="c5")
    nc.vector.scalar_tensor_tensor(c5[:], in0=stat_xasq[:], scalar=0.2,
                                   in1=c4[:], op0=ALU.mult, op1=ALU.add)
    lr = stats.tile([P, 1], f32, name="lr")
    nc.vector.tensor_reduce(lr[:], c5[:], axis=mybir.AxisListType.X, op=ALU.add)
    lsc = stats.tile([P, 1], f32, name="lsc")
    nc.vector.tensor_scalar_mul(lsc[:], lr[:], 0.5 / B)
    nc.sync.dma_start(out_ap[:], lsc[:])


def _build():
    if "nc" in _COMPILED:
        return _COMPILED["nc"]
    nc = bacc.Bacc("TRN2", target_bir_lowering=False, debug=False)
    xt_d = nc.dram_tensor("xt", [K, BSH], bf16, kind="ExternalInput").ap()
    y_d = nc.dram_tensor("y", [BSH, D], f32, kind="ExternalInput").ap()
    a_d = nc.dram_tensor("a", [K, D], bf16, kind="ExternalInput").ap()
    out_d = nc.dram_tensor("out", [P, 1], f32, kind="ExternalOutput").ap()
    with tile.TileContext(nc) as tc:
        _loss_kernel(tc, out_d, xt_d, y_d, a_d)
    nc.compile()
    _COMPILED["nc"] = nc
    return nc


def kernel(A, y, x, _trace=False):
    nc = _build()
    a_bf = np.asarray(A, dtype=np.float32).astype(ml_dtypes.bfloat16)
    in_maps = []
    for c in range(NCORES):
        sl = slice(c * BSH, (c + 1) * BSH)
        xt_c = np.ascontiguousarray(np.asarray(x[sl], dtype=np.float32).T).astype(
            ml_dtypes.bfloat16)
        y_c = np.ascontiguousarray(np.asarray(y[sl], dtype=np.float32))
        in_maps.append({"xt": xt_c, "y": y_c, "a": a_bf})
    try:
        res = bass_utils.run_bass_kernel_spmd(
            nc, in_maps, core_ids=list(range(NCORES)), trace=_trace)
    except ModuleNotFoundError:
        res = bass_utils.run_bass_kernel_spmd(
            nc, in_maps, core_ids=list(range(NCORES)), trace=False)
    total = 0.0
    for c in range(NCORES):
        total += res.results[c]["out"].astype(np.float64).sum()
    out = np.float32(total)
    if _trace:
        return out, res
    return out
